# revision 1
# baseline (speedup 1.0000x reference)
"""Trainium2 Bass kernel for nn_CrossAttentionDown (region-RoPE cross attention).

Full-input contract: kernel(**inputs) takes the complete tensors, shards
(B, H) across 8 NeuronCores (each core: one batch, half the heads), runs an
SPMD Bass kernel, and gathers the full [B, H, P, D] output.

Math notes (vs the jax reference):
 - softmax(x + c) == softmax(x) per row, so the per-head bias_diff constant
   drops out; only delta_h = bias_same - bias_diff matters. It is folded into
   the QK^T contraction via 32 extra dims: K side gets onehot(regions[t]==n)
   (written once per core), Q side gets delta_h * onehot(n == p//4).
 - scores are computed transposed ([t, p] layout) so both the exp output and
   V can feed the AV matmul with t on the contraction (partition) dim. The
   AV matmul uses exp(scores) tiles as the stationary operand, so the output
   lands directly as [p, d]; the softmax denominator is one extra
   ones-column matmul sharing the same stationary tile.
 - tokens are tiled contiguously across partitions (token = 32*p + jj) so
   K/V DMA reads 8KB contiguous per partition; attention is invariant to
   the token permutation as long as K, V, regions and the rope/bias tables
   use the same ordering.
 - rope on K: the global-position half uses compile-time cos/sin tables
   (positions are static); the region half is built per core from regions
   via a short range-reduce + Sin chain. The rotation itself is
   out = k*chat + pairswap(k)*shat with all-fp16 packed operands (DVE 2x),
   where pairswap is a negative-stride access-pattern view.
 - region starts (first t with regions==n, 0 if absent) are computed on
   device: per-region counts via a tensor_tensor_reduce accumulator, then
   an exclusive prefix sum via a constant triangular matmul (regions are
   sorted).
"""

import sys

if "/opt/trn_rl_repo" not in sys.path:
    sys.path.insert(0, "/opt/trn_rl_repo")

import math

import numpy as np

B, H, T, D = 4, 16, 4096, 64
MAX_N = 32
R_TOK = 4
P = MAX_N * R_TOK  # 128 pool queries
NCORES = 8
HPC = H // 2  # heads per core
NT = T // 128  # 32 token tiles of 128
TPP = T // 128  # tokens per partition = 32
NPAIR = 16  # rotation pairs per half (half dim = 32)
KAUG = D + MAX_N  # 96 contraction dims (64 rot + 32 bias one-hot)
THETA = 10000.0

_cache = {}


def _split_waits(nc, maxw=1):
    """The pinned walrus rejects instructions with more than one embedded
    semaphore wait. Hoist excess waits into preceding wait-only Drain
    instructions on the same engine (same-engine program order preserves
    the blocking semantics)."""
    import concourse.mybir as mybir

    n_new = 0
    for f in nc.m.functions:
        for blk in f.blocks:
            new_list = []
            for inst in blk.instructions:
                si = getattr(inst, "sync_info", None)
                waits = list(si.on_wait) if si is not None and si.on_wait else []
                if len(waits) > maxw:
                    excess, keep = waits[:-maxw], waits[-maxw:]
                    for j, w in enumerate(excess):
                        d = mybir.InstDrain(name=f"{inst.name}-w{j}", ins=[], outs=[])
                        d.engine = inst.engine
                        d.sync_info = mybir.SyncInfo(on_wait=[w], on_update=[])
                        d.debug = inst.debug
                        new_list.append(d)
                        n_new += 1
                    si.on_wait = keep
                new_list.append(inst)
            blk.instructions[:] = new_list
    return n_new


def _emit_range_reduce(nc, mybir, pool, ang, ncols, name):
    """In-place reduce ang (>=0, < ~2^20) to [-pi, pi] mod 2pi. k is computed
    with the fp32 magic-number trick (guaranteed round-to-nearest), then a
    two-term Cody-Waite subtraction (hi=6.28125, k*hi exact for small k),
    then a clamp for boundary epsilon."""
    f32 = mybir.dt.float32
    INV2PI = float(np.float32(1.0 / (2.0 * math.pi)))
    HI = 6.28125
    LO = float(np.float32(2.0 * math.pi - HI))
    PI = float(np.float32(math.pi))
    MAGIC = float(np.float32(1.5 * 2.0**23))
    kf = pool.tile([128, ncols], f32, name=f"{name}_kf", tag=f"{name}_kf")
    nc.vector.tensor_scalar(
        kf[:], ang, INV2PI, MAGIC, op0=mybir.AluOpType.mult, op1=mybir.AluOpType.add
    )
    nc.vector.tensor_scalar_add(kf[:], kf[:], -MAGIC)
    nc.vector.scalar_tensor_tensor(
        ang, kf[:], -HI, ang, op0=mybir.AluOpType.mult, op1=mybir.AluOpType.add
    )
    nc.vector.scalar_tensor_tensor(
        ang, kf[:], -LO, ang, op0=mybir.AluOpType.mult, op1=mybir.AluOpType.add
    )
    mt = pool.tile([128, ncols], f32, name=f"{name}_mt", tag=f"{name}_mt")
    nc.vector.tensor_scalar(mt[:], ang, PI, None, op0=mybir.AluOpType.is_gt)
    nc.vector.scalar_tensor_tensor(
        ang, mt[:], -(HI + LO), ang,
        op0=mybir.AluOpType.mult, op1=mybir.AluOpType.add,
    )
    nc.vector.tensor_scalar(mt[:], ang, -PI, None, op0=mybir.AluOpType.is_lt)
    nc.vector.scalar_tensor_tensor(
        ang, mt[:], (HI + LO), ang,
        op0=mybir.AluOpType.mult, op1=mybir.AluOpType.add,
    )


def _emit_sincos(nc, mybir, pool, presb_tile_fn, ang, ncols, name, AF, halfpi):
    """Given ang in [-pi, pi], produce (sin, cos) tiles; cos = sin of the
    +pi/2-shifted, re-wrapped angle (clobbers ang)."""
    import math as _math

    f32 = mybir.dt.float32
    sin_t = presb_tile_fn([128, ncols], f32, f"{name}_sin")
    nc.scalar.activation(sin_t[:], ang, AF.Sin)
    nc.vector.tensor_scalar_add(ang, ang, float(_math.pi / 2))
    mt = presb_tile_fn([128, ncols], f32, f"{name}_mt2")
    nc.vector.tensor_scalar(
        mt[:], ang, float(np.float32(_math.pi)), None, op0=mybir.AluOpType.is_gt
    )
    nc.vector.scalar_tensor_tensor(
        ang, mt[:], float(-2.0 * _math.pi), ang,
        op0=mybir.AluOpType.mult, op1=mybir.AluOpType.add,
    )
    cos_t = presb_tile_fn([128, ncols], f32, f"{name}_cos")
    nc.scalar.activation(cos_t[:], ang, AF.Sin)
    return sin_t, cos_t


def _build_program():
    import concourse.bass as bass
    import concourse.mybir as mybir
    import concourse.tile as tile

    f32 = mybir.dt.float32
    f16 = mybir.dt.float16  # 16-bit matmul dtype (fp16: 11-bit mantissa)
    AF = mybir.ActivationFunctionType

    nc = bass.Bass("TRN2", target_bir_lowering=False, debug=False)

    q_d = nc.dram_tensor("q", [HPC, P, D], f32, kind="ExternalInput")
    k_d = nc.dram_tensor("k", [HPC, T, D], f32, kind="ExternalInput")
    v_d = nc.dram_tensor("v", [HPC, T, D], f32, kind="ExternalInput")
    reg_d = nc.dram_tensor("regions_f", [T], f32, kind="ExternalInput")
    bs_d = nc.dram_tensor("bias_same8", [HPC], f32, kind="ExternalInput")
    bd_d = nc.dram_tensor("bias_diff8", [HPC], f32, kind="ExternalInput")
    out_d = nc.dram_tensor("out", [HPC, P, D], f32, kind="ExternalOutput")

    # ---- compile-time constants ----
    inv = (1.0 / (THETA ** (np.arange(0, 32, 2, dtype=np.float64) / 32.0))).astype(
        np.float32
    )  # [16] rope inverse freqs (per half, half dim 32)
    # token(p, jj) = 32*p + jj ; position-half cos/sin, expanded to pairs
    tok = (32.0 * np.arange(128, dtype=np.float64)[:, None]
           + np.arange(TPP, dtype=np.float64)[None, :])  # [128, 32]
    ang1 = tok[:, :, None] * inv.astype(np.float64)[None, None, :]  # [128,32,16]
    c1_np = np.repeat(np.cos(ang1), 2, axis=-1).reshape(128, TPP * 32)
    s1_half = np.sin(ang1)
    s1_np = np.stack([-s1_half, s1_half], axis=-1).reshape(128, TPP * 32)
    c1_np = c1_np.astype(np.float16)
    s1_np = s1_np.astype(np.float16)

    ident_np = np.eye(128, dtype=np.float16)
    inv128_np = np.broadcast_to(inv[None, :], (128, NPAIR)).copy()
    nvals_np = np.arange(1, MAX_N + 1, dtype=np.float32)[:, None].copy()  # [32,1]
    onehotP_np = (
        np.arange(MAX_N)[:, None] == (np.arange(128)[None, :] // R_TOK)
    ).astype(np.float32)
    atpre_np = (
        np.arange(MAX_N)[:, None] < (np.arange(128)[None, :] // R_TOK)
    ).astype(np.float32)
    ridx_np = (np.arange(128, dtype=np.float32) // R_TOK + 1.0)[:, None].copy()

    # merged const blocks (one DMA each — HWDGE costs ~625ns per DMA instr)
    ones16_np = np.ones((128, 1), np.float16)
    cfull_np = np.zeros((128, TPP, 64), np.float16)
    cfull_np[:, :, 0:32] = c1_np.reshape(128, TPP, 32)
    sfull_np = np.zeros((128, TPP, 64), np.float16)
    sfull_np[:, :, 0:32] = s1_np.reshape(128, TPP, 32)
    blk16_np = np.concatenate(
        [ident_np, cfull_np.reshape(128, TPP * 64), sfull_np.reshape(128, TPP * 64),
         ones16_np], axis=1)  # [128, 4225]
    nids128_np = np.broadcast_to(
        np.arange(1, MAX_N + 1, dtype=np.float32)[None, :], (128, MAX_N)
    ).copy()
    halfpi_np = np.full((128, 1), math.pi / 2.0, np.float32)
    blkA_np = np.concatenate([inv128_np, ridx_np, nids128_np, halfpi_np], axis=1)  # [128, 50]
    blkB_np = np.concatenate([nvals_np, onehotP_np, atpre_np], axis=1)  # [32, 257]

    blk16_c = nc.inline_tensor(blk16_np, name="blk16_c")
    blkA_c = nc.inline_tensor(blkA_np, name="blkA_c")
    blkB_c = nc.inline_tensor(blkB_np, name="blkB_c")

    with tile.TileContext(nc) as tc:
        with tc.tile_pool(name="const", bufs=1) as cpool:
            blk16 = cpool.tile([128, 4225], f16, name="blk16")
            blkA = cpool.tile([128, 50], f32, name="blkA")
            blkB = cpool.tile([MAX_N, 257], f32, name="blkB")
            ident = blk16[:, 0:128]
            cfull = blk16[:, 128 : 128 + TPP * 64]
            sfull = blk16[:, 128 + TPP * 64 : 128 + 2 * TPP * 64]
            inv128 = blkA[:, 0:NPAIR]
            ridx = blkA[:, NPAIR : NPAIR + 1]
            onesf16 = blk16[:, 4224:4225]
            halfpi = blkA[:, 49:50]
            nvals = blkB[:, 0:1]
            onehotP = blkB[:, 1:129]
            atpre = blkB[:, 129:257]

            with tc.tile_pool(name="tables", bufs=1) as tpool:
                # persistent per-core tables
                kta = tpool.tile([KAUG, 2 * T], f16, name="kta")  # double-buffered by head parity
                qT_all = tpool.tile([KAUG, HPC * 128], f16, name="qT_all")
                gpos = tpool.tile([128, 1], f32, name="gpos")
                delta32 = tpool.tile([MAX_N, HPC], f32, name="delta32")

                # DMA issue order (HWDGE generates descriptors serially at
                # ~625ns/DMA): head-0 K/V first so transfers start at t~1us,
                # then the preamble inputs + merged const blocks (the rotate
                # tables depend on them), then the remaining K/V stream.
                # Out-DMAs go on the ACT queue so they cannot
                # head-of-line-block these.
                prio = tc.alloc_tile_pool(name="pre_io", bufs=1)
                iopool = tc.alloc_tile_pool(name="io", bufs=3)
                ksbs, vsbs = [], []
                for h in range(HPC):
                    ksbs.append(iopool.tile([128, TPP * D], f32, name="ksb", tag="ksb"))
                    vsbs.append(iopool.tile([128, TPP * D], f32, name="vsb", tag="vsb"))

                def _kv_dma(h):
                    nc.sync.dma_start(
                        ksbs[h].rearrange("p (t d) -> p t d", t=TPP),
                        k_d.ap()[h].rearrange("(p t) d -> p t d", t=TPP),
                    )
                    nc.sync.dma_start(
                        vsbs[h].rearrange("p (t d) -> p t d", t=TPP),
                        v_d.ap()[h].rearrange("(p t) d -> p t d", t=TPP),
                    )

                nc.sync.dma_start(
                    ksbs[0].rearrange("p (t d) -> p t d", t=TPP),
                    k_d.ap()[0].rearrange("(p t) d -> p t d", t=TPP),
                )
                regf = prio.tile([128, TPP], f32, name="regf")
                nc.sync.dma_start(regf[:], reg_d.ap().rearrange("(p t) -> p t", t=TPP))
                nc.sync.dma_start(blkA[:], blkA_c.ap())
                nc.sync.dma_start(blkB[:], blkB_c.ap())
                nc.sync.dma_start(blk16[:], blk16_c.ap())
                nc.sync.dma_start(
                    vsbs[0].rearrange("p (t d) -> p t d", t=TPP),
                    v_d.ap()[0].rearrange("(p t) d -> p t d", t=TPP),
                )
                qsb = prio.tile([128, HPC * D], f32, name="qsb")
                nc.sync.dma_start(
                    qsb.rearrange("p (h d) -> p h d", h=HPC),
                    q_d.ap().rearrange("h p d -> p h d"),
                )
                bs_sb = prio.tile([MAX_N, HPC], f32, name="bs_sb")
                bd_sb = prio.tile([MAX_N, HPC], f32, name="bd_sb")
                nc.sync.dma_start(
                    bs_sb[:],
                    bs_d.ap().rearrange("(o h) -> o h", o=1).broadcast_to([MAX_N, HPC]),
                )
                nc.sync.dma_start(
                    bd_sb[:],
                    bd_d.ap().rearrange("(o h) -> o h", o=1).broadcast_to([MAX_N, HPC]),
                )
                for h in range(1, HPC):
                    _kv_dma(h)

                # main-loop pools allocated BEFORE the preamble scratch so
                # the pipeline's SBUF/PSUM does not alias preamble tiles
                # (aliasing would serialize the first heads behind preamble
                # readers). Preamble matmuls borrow pipeline PSUM tiles.
                wpool = tc.alloc_tile_pool(name="work", bufs=3)
                vpool = tc.alloc_tile_pool(name="vwork", bufs=4)
                apool = tc.alloc_tile_pool(name="attn", bufs=4)
                fpool = tc.alloc_tile_pool(name="fin", bufs=2)
                ktps = tc.alloc_tile_pool(name="kt_ps", bufs=2, space="PSUM")
                scps = tc.alloc_tile_pool(name="sc_ps", bufs=2, space="PSUM")
                avps = tc.alloc_tile_pool(name="av_ps", bufs=2, space="PSUM")

                with tc.tile_pool(name="pre_sb", bufs=1) as presb:
                    # region-half angles: the critical intro chain (head 0's
                    # rotate waits on c2/s2), so scheduled at high priority
                    with tc.high_priority():
                        angr = presb.tile([128, TPP * NPAIR], f32, name="angr")
                        nc.vector.tensor_mul(
                            angr[:].rearrange("p (t j) -> p t j", j=NPAIR),
                            regf[:, :, None].broadcast_to([128, TPP, NPAIR]),
                            blkA[:, None, 0:NPAIR].broadcast_to([128, TPP, NPAIR]),
                        )
                        _emit_range_reduce(nc, mybir, presb, angr[:], TPP * NPAIR, "rrk")
                        sinr, cosr = _emit_sincos(
                            nc, mybir, presb,
                            lambda s, d, n: presb.tile(s, d, name=n),
                            angr[:], TPP * NPAIR, "rrk", AF, halfpi,
                        )

                    # per-region counts via a small [p, (jj, n)] onehot and
                    # 32 accumulating matmuls with a ones vector (cheap and
                    # early — only needs regf — feeds the whole Q chain)
                    oh = presb.tile([128, TPP * MAX_N], f16, name="oh")
                    oh_v = oh.rearrange("p (t n) -> p t n", n=MAX_N)
                    nc.vector.tensor_tensor(
                        oh_v,
                        regf[:, :, None].broadcast_to([128, TPP, MAX_N]),
                        blkA[:, None, 17:49].broadcast_to([128, TPP, MAX_N]),
                        op=mybir.AluOpType.is_equal,
                    )
                    cnt_ps = avps.tile([128, D + 1], f32, name="avp", tag="avp")
                    for t in range(TPP):
                        nc.tensor.matmul(
                            cnt_ps[0:MAX_N, 0:1],
                            oh_v[:, t, :],
                            onesf16,
                            start=(t == 0),
                            stop=(t == TPP - 1),
                        )
                    cnt_sb = presb.tile([MAX_N, 1], f32, name="cnt_sb")
                    nc.vector.tensor_copy(cnt_sb[:], cnt_ps[0:MAX_N, 0:1])

                    # starts (exclusive prefix over counts) -> gpos
                    st_ps = avps.tile([128, D + 1], f32, name="avp", tag="avp")
                    nc.tensor.matmul(st_ps[:, 0:1], atpre, cnt_sb[:], start=True, stop=True)
                    nc.tensor.matmul(st_ps[:, 4:5], onehotP, cnt_sb[:], start=True, stop=True)
                    gtm = presb.tile([128, 1], f32, name="gtm")
                    nc.vector.tensor_scalar(
                        gtm[:], st_ps[:, 4:5], 0.0, None, op0=mybir.AluOpType.is_gt
                    )
                    nc.vector.tensor_mul(gpos[:], st_ps[:, 0:1], gtm[:])

                    # Q-side angles (after gpos)
                    angq = presb.tile([128, 2 * NPAIR], f32, name="angq")
                    nc.vector.tensor_scalar_mul(angq[:, 0:NPAIR], inv128, gpos[:])
                    nc.vector.tensor_scalar_mul(angq[:, NPAIR : 2 * NPAIR], inv128, ridx)
                    _emit_range_reduce(nc, mybir, presb, angq[:], 2 * NPAIR, "rrq")
                    sinq, cosq = _emit_sincos(
                        nc, mybir, presb,
                        lambda s, d, n: presb.tile(s, d, name=n),
                        angq[:], 2 * NPAIR, "rrq", AF, halfpi,
                    )

                    # Q cos/sin expansion with the 1/8 score scale (ACT) —
                    # emitted before the c2/s2 expansion so the Q chain
                    # (qrot on DVE) unblocks as early as possible
                    cq = presb.tile([128, D], f32, name="cq")
                    sq = presb.tile([128, D], f32, name="sq")
                    cq_v = cq.rearrange("p (j e) -> p j e", e=2)
                    sq_v = sq.rearrange("p (j e) -> p j e", e=2)
                    nc.scalar.mul(
                        cq_v, cosq[:, :, None].broadcast_to([128, 2 * NPAIR, 2]), 0.125
                    )
                    nc.scalar.mul(sq_v[:, :, 1], sinq[:], 0.125)
                    nc.scalar.mul(sq_v[:, :, 0], sinq[:], -0.125)

                    # region-half tables c2/s2 (pair-slot expansion, on ACT)
                    with tc.high_priority():
                        c2_v = cfull.rearrange("p (t c) -> p t c", t=TPP)[
                            :, :, 32:64
                        ].rearrange("p t (j e) -> p t j e", e=2)
                        s2_v = sfull.rearrange("p (t c) -> p t c", t=TPP)[
                            :, :, 32:64
                        ].rearrange("p t (j e) -> p t j e", e=2)
                        cr_v = cosr[:, :, None].rearrange("p (t j) e -> p t j e", j=NPAIR)
                        sr_v = sinr.rearrange("p (t j) -> p t j", j=NPAIR)
                        nc.scalar.activation(
                            c2_v, cr_v.broadcast_to([128, TPP, NPAIR, 2]), AF.Copy
                        )
                        nc.scalar.activation(s2_v[:, :, :, 1], sr_v, AF.Copy)
                        nc.scalar.mul(s2_v[:, :, :, 0], sr_v, -1.0)

                    # rotate all q heads: qrot = q*cq + swap(q)*sq
                    qs_v = qsb.rearrange("p (h d) -> p h d", h=HPC)
                    qs_swap = qsb.rearrange("p (h j e) -> p h j e", h=HPC, e=2)[
                        :, :, :, ::-1
                    ]
                    qrot = presb.tile([128, HPC * D], f16, name="qrot")
                    qtm = presb.tile([128, HPC * D], f16, name="qtm")
                    qr_v = qrot.rearrange("p (h d) -> p h d", h=HPC)
                    sq_v4 = sq.rearrange("p (j e) -> p j e", e=2)[:, None, :, :]
                    nc.vector.tensor_mul(
                        qr_v, qs_v, cq[:, None, :].broadcast_to([128, HPC, D])
                    )
                    nc.vector.tensor_mul(
                        qtm.rearrange("p (h j e) -> p h j e", h=HPC, e=2),
                        qs_swap,
                        sq_v4.broadcast_to([128, HPC, 2 * NPAIR, 2]),
                    )
                    nc.vector.tensor_add(qrot[:], qrot[:], qtm[:])

                    # transpose q (8 heads) into one psum bank, copy once
                    qtp = ktps.tile([128, 1024], f16, name="ktp", tag="ktp")
                    qr_h = qrot.rearrange("p (h d) -> p h d", h=HPC)
                    for h in range(HPC):
                        nc.tensor.transpose(
                            qtp[0:D, h * 128 : (h + 1) * 128], qr_h[:, h, :], ident
                        )
                    nc.vector.tensor_copy(qT_all[0:D, :], qtp[0:D, :])

                    # bias rows: qT[64+n, h*128+p] = delta[n, h] * onehotP[n, p]
                    nc.vector.tensor_sub(delta32[:], bs_sb[:], bd_sb[:])
                    qb_v = qT_all[D:KAUG, :].rearrange("n (h p) -> n h p", h=HPC)
                    nc.vector.tensor_mul(
                        qb_v,
                        delta32[:, :, None].broadcast_to([MAX_N, HPC, 128]),
                        blkB[:, None, 1:129].broadcast_to([MAX_N, HPC, 128]),
                    )

                    # kta onehot rows 64:96 buf0: transpose the [p, jj, n]
                    # onehot tiles on PE (col t*128+q of kta holds token
                    # 32q+t = oh[q, t, n] transposed); the buf1 copy goes on
                    # ACT at the top of head 1
                    for g in range(4):
                        ohp = ktps.tile([128, 1024], f16, name="ktp", tag="ktp")
                        for i in range(8):
                            jj = g * 8 + i
                            nc.tensor.transpose(
                                ohp[0:MAX_N, i * 128 : (i + 1) * 128],
                                oh_v[:, jj, :],
                                ident,
                            )
                        nc.scalar.activation(
                            kta[D:KAUG, g * 1024 : (g + 1) * 1024], ohp[0:MAX_N, :],
                            AF.Copy,
                        )

                # ================= main per-head loop =================
                for h in range(HPC):
                    kb = (h % 2) * T  # kta column base for this head
                    ksb, vsb = ksbs[h], vsbs[h]

                    # f32 -> fp16 casts on gpsimd (otherwise idle);
                    # vbf keeps a ones column per tile for the softmax
                    # denominator (written once per parity buffer)
                    kbf = wpool.tile([128, TPP * D], f16, name="kbf", tag="kbf")
                    nc.gpsimd.tensor_copy(kbf[:], ksb[:])
                    vbf = vpool.tile([128, TPP * (D + 1)], f16, name="vbf", tag="vbf")
                    vb_t = vbf.rearrange("p (t d) -> p t d", t=TPP)
                    nc.gpsimd.tensor_copy(
                        vb_t[:, :, 0:D], vsb.rearrange("p (t d) -> p t d", t=TPP)
                    )
                    nc.gpsimd.memset(vb_t[:, :, D : D + 1], 1.0)
                    if h == 1:
                        nc.scalar.activation(
                            kta[D:KAUG, T : 2 * T], kta[D:KAUG, 0:T], AF.Copy
                        )

                    # rotate K: kra = kbf*c + pairswap(kbf)*s (all fp16,
                    # packed operands -> DVE 2x rate; HW transpose-mode
                    # cannot accumulate in PSUM, so the add happens here)
                    kra = wpool.tile([128, TPP * D], f16, name="kra", tag="kra")
                    tmp = wpool.tile([128, TPP * D], f16, name="tmp", tag="tmp")
                    ksw = kbf.rearrange("p (t j e) -> p t j e", t=TPP, e=2)[
                        :, :, :, ::-1
                    ]
                    nc.vector.tensor_mul(kra[:], kbf[:], cfull)
                    nc.vector.tensor_mul(
                        tmp.rearrange("p (t j e) -> p t j e", t=TPP, e=2),
                        ksw,
                        sfull.rearrange("p (t j e) -> p t j e", t=TPP, e=2),
                    )
                    nc.vector.tensor_add(kra[:], kra[:], tmp[:])

                    # transpose: 2 tiles per [128,128] PE transpose, 8 pairs
                    # per psum group; unpack even/odd tiles with strided
                    # copies (one of them on ACT to balance DVE)
                    for g in range(2):
                        ktp = ktps.tile([128, 1024], f16, name="ktp", tag="ktp")
                        for i in range(8):
                            t2 = g * 8 + i  # covers k-tiles 2*t2, 2*t2+1
                            nc.tensor.transpose(
                                ktp[:, i * 128 : (i + 1) * 128],
                                kra[:, (2 * t2) * D : (2 * t2 + 2) * D],
                                ident,
                            )
                        kta_g = kta[0:D, kb + g * 2048 : kb + (g + 1) * 2048]
                        kta_v = kta_g.rearrange("c (i e o) -> c i e o", i=8, e=2)
                        ktp_e = ktp[0:D, :].rearrange("c (i o) -> c i o", i=8)
                        ktp_o = ktp[D:128, :].rearrange("c (i o) -> c i o", i=8)
                        nc.vector.tensor_copy(kta_v[:, :, 0, :], ktp_e)
                        nc.vector.tensor_copy(kta_v[:, :, 1, :], ktp_o)

                    # scores (transposed), exp, AV accumulation
                    at = apool.tile([128, T], f16, name="at", tag="at")
                    avp = avps.tile([128, D + 1], f32, name="avp", tag="avp")
                    for g in range(4):
                        scp = scps.tile([128, 1024], f32, name="scp", tag="scp")
                        for i in range(8):
                            t = g * 8 + i
                            nc.tensor.matmul(
                                scp[:, i * 128 : (i + 1) * 128],
                                kta[0:KAUG, kb + t * 128 : kb + (t + 1) * 128],
                                qT_all[0:KAUG, h * 128 : (h + 1) * 128],
                                start=True,
                                stop=True,
                            )
                        nc.scalar.activation(
                            at[:, g * 1024 : (g + 1) * 1024], scp[:], AF.Exp
                        )
                    # AV after all score groups: exp(g) overlaps scores(g+1)
                    # instead of stalling the PE queue behind each exp
                    for t in range(NT):
                        nc.tensor.matmul(
                            avp[:],
                            at[:, t * 128 : (t + 1) * 128],
                            vbf[:, t * (D + 1) : (t + 1) * (D + 1)],
                            start=(t == 0),
                            stop=(t == NT - 1),
                        )

                    # epilogue: normalize by the ones-column sum, store
                    rden = fpool.tile([128, 1], f32, name="rden", tag="rden")
                    nc.vector.reciprocal(rden[:], avp[:, D : D + 1])
                    osb = fpool.tile([128, D], f32, name="osb", tag="osb")
                    nc.scalar.activation(
                        osb[:], avp[:, 0:D], AF.Copy, scale=rden[:]
                    )
                    nc.scalar.dma_start(out_d.ap()[h], osb[:])
                # release in reverse allocation (stack) order
                for _p in (avps, scps, ktps, fpool, apool, vpool, wpool, iopool, prio):
                    _p.release()

    _split_waits(nc)
    return nc


def _get_program():
    if "nc" not in _cache:
        _cache["nc"] = _build_program()
    return _cache["nc"]


def _make_in_maps(query_q, x_k, x_v, regions, bias_same, bias_diff):
    query_q = np.asarray(query_q, dtype=np.float32)
    x_k = np.asarray(x_k, dtype=np.float32)
    x_v = np.asarray(x_v, dtype=np.float32)
    regions_f = np.asarray(regions).astype(np.float32)
    bias_same = np.asarray(bias_same, dtype=np.float32)
    bias_diff = np.asarray(bias_diff, dtype=np.float32)

    in_maps = []
    for core in range(NCORES):
        b = core // 2
        h0 = (core % 2) * HPC
        in_maps.append(
            {
                "q": np.ascontiguousarray(query_q[b, h0 : h0 + HPC]),
                "k": np.ascontiguousarray(x_k[b, h0 : h0 + HPC]),
                "v": np.ascontiguousarray(x_v[b, h0 : h0 + HPC]),
                "regions_f": np.ascontiguousarray(regions_f[b]),
                "bias_same8": np.ascontiguousarray(bias_same[h0 : h0 + HPC]),
                "bias_diff8": np.ascontiguousarray(bias_diff[h0 : h0 + HPC]),
            }
        )
    return in_maps


def _gather(res):
    out = np.empty((B, H, P, D), np.float32)
    for core in range(NCORES):
        b = core // 2
        h0 = (core % 2) * HPC
        out[b, h0 : h0 + HPC] = res.results[core]["out"]
    return out


def kernel(
    query_q,
    x_k,
    x_v,
    regions,
    t_mask=None,
    n_mask=None,
    max_n=None,
    bias_same=None,
    bias_diff=None,
    **_unused,
):
    from concourse import bass_utils

    nc = _get_program()
    in_maps = _make_in_maps(query_q, x_k, x_v, regions, bias_same, bias_diff)
    res = bass_utils.run_bass_kernel_spmd(nc, in_maps, core_ids=list(range(NCORES)))
    return _gather(res)


def hw_bench(inputs, m_small=8, m_big=64, reps=3):
    """Estimate per-execution device time: async-dispatch M independent
    executions of the NEFF (device serializes them), block once, and take
    the marginal wall time between M=m_small and M=m_big."""
    import time

    import jax
    import jax.numpy as jnp
    from jax.sharding import Mesh, PartitionSpec
    from jax.experimental.shard_map import shard_map

    from concourse import bass2jax, mybir

    bass2jax.install_neuronx_cc_hook()
    nc = _get_program()
    in_maps = _make_in_maps(
        inputs["query_q"], inputs["x_k"], inputs["x_v"], inputs["regions"],
        inputs["bias_same"], inputs["bias_diff"],
    )

    partition_name = (
        nc.partition_id_tensor.name if nc.partition_id_tensor else None
    )
    in_names, out_names, out_avals = [], [], []
    for alloc in nc.m.functions[0].allocations:
        if not isinstance(alloc, mybir.MemoryLocationSet):
            continue
        name = alloc.memorylocations[0].name
        if alloc.kind == "ExternalInput":
            if name != partition_name:
                in_names.append(name)
        elif alloc.kind == "ExternalOutput":
            out_names.append(name)
            out_avals.append(
                jax.core.ShapedArray(
                    tuple(alloc.tensor_shape), mybir.dt.np(alloc.dtype)
                )
            )
    all_in_names = tuple(
        in_names + out_names + ([partition_name] if partition_name else [])
    )

    def _body(*args):
        operands = list(args)
        if partition_name:
            operands.append(bass2jax.partition_id_tensor())
        return tuple(
            bass2jax._bass_exec_p.bind(
                *operands,
                out_avals=tuple(out_avals),
                in_names=all_in_names,
                out_names=tuple(out_names),
                lowering_input_output_aliases=(),
                sim_require_finite=True,
                sim_require_nnan=True,
                nc=nc,
            )
        )

    devices = jax.devices()[:NCORES]
    mesh = Mesh(np.asarray(devices), ("core",))
    n_in = len(in_names)
    n_out = len(out_names)
    sharded = jax.jit(
        shard_map(
            _body,
            mesh=mesh,
            in_specs=(PartitionSpec("core"),) * (n_in + n_out),
            out_specs=(PartitionSpec("core"),) * n_out,
            check_rep=False,
        )
    )
    per_core = [[np.asarray(m[nm]) for nm in in_names] for m in in_maps]
    concat_in = [
        np.concatenate([per_core[c][i] for c in range(NCORES)], axis=0)
        for i in range(n_in)
    ]
    zeros = [
        np.zeros((NCORES * a.shape[0], *a.shape[1:]), a.dtype) for a in out_avals
    ]
    dev_args = [jax.device_put(a) for a in concat_in + zeros]

    # warm (compile + first exec)
    jax.block_until_ready(sharded(*dev_args))

    def _run(m):
        best = float("inf")
        for _ in range(reps):
            outs = None
            t0 = time.perf_counter()
            for _i in range(m):
                outs = sharded(*dev_args)
            jax.block_until_ready(outs)
            best = min(best, time.perf_counter() - t0)
        return best

    t_small = _run(m_small)
    t_big = _run(m_big)
    per_exec_ns = (t_big - t_small) / (m_big - m_small) * 1e9
    return per_exec_ns, t_small, t_big

